# revision 11
# baseline (speedup 1.0000x reference)
"""AttentionPooling (segment softmax + weighted segment-sum) Trainium2 kernel.

Algorithm (reference without explicit seg_max subtraction — scores are tiny,
|s| < ~3, so exp() is numerically safe unshifted and softmax is
shift-invariant):

    s_i   = W2^T prelu(W1^T x_i + b1) + b2          (per node)
    e_i   = exp(s_i)
    out_g = (sum_{i in g} e_i x_i) / (sum_{i in g} e_i + 1e-16)

Sharding: 16384 segments -> 8 cores x 16 blocks x 128 segments. batch is
sorted, so each (core, block) owns a contiguous node range; host pads every
block to the same number K of 128-node tiles so one SPMD program serves all
cores.

The host supplies each 128-node tile in BOTH orientations (bf16):
  - xT  [D=128, nodes]   feeds mm1 directly (no on-device transpose)
  - xn  [nodes, 1+D]     col 0 = 1.0; one DVE tensor_scalar produces the
                         pooling rhs [e | e*x] in a single op
Per tile: mm1 (PE) -> prelu (ACT, same table set as Exp) -> mm2 (PE, scores
column) -> exp (ACT) -> xe (DVE) -> one-hot pooling matmul (PE) accumulated
per 128-segment block in PSUM; block flush = reciprocal + scale + DMA out.
"""

import os
import numpy as np
import ml_dtypes

N = 2_000_000
D = 128
H = 64
G = 16384
NEG_SLOPE = 0.01
NCORES = 8
SEGS_PER_CORE = G // NCORES          # 2048
SEGS_PER_BLOCK = 128
BLOCKS_PER_CORE = SEGS_PER_CORE // SEGS_PER_BLOCK   # 16
TILE_N = 128
GROUP = 8                            # tiles per group (ACT chunking)
CW = D + 1                           # natural-tile width incl. ones column

_bf16 = ml_dtypes.bfloat16


def _host_prep(x, batch, W1, b1, W2, b2):
    bounds = np.searchsorted(batch, np.arange(0, G + 1, SEGS_PER_BLOCK))
    cnts = np.diff(bounds)
    K = int(np.max((cnts + TILE_N - 1) // TILE_N))
    K = ((K + GROUP - 1) // GROUP) * GROUP
    NT = BLOCKS_PER_CORE * K
    NG = NT // GROUP

    in_maps = []
    for c in range(NCORES):
        xn = np.zeros((NT, TILE_N, CW), dtype=np.float32)   # [tile, node, 1+D]
        bl = np.full((NT * TILE_N,), -1.0, dtype=np.float32)
        for b in range(BLOCKS_PER_CORE):
            gb = c * BLOCKS_PER_CORE + b
            lo, hi = int(bounds[gb]), int(bounds[gb + 1])
            n = hi - lo
            base = b * K * TILE_N
            blk = np.zeros((K * TILE_N, D), dtype=np.float32)
            blk[:n] = x[lo:hi]
            xn[b * K:(b + 1) * K, :, 1:] = blk.reshape(K, TILE_N, D)
            xn[b * K:(b + 1) * K, :, 0] = 1.0
            bl[base:base + n] = (batch[lo:hi] - gb * SEGS_PER_BLOCK).astype(np.float32)
        xn16 = xn.astype(_bf16)
        # xT groups: [NG*D, GROUP*TILE_N], row p = concat_j tileT(g*8+j)[p, :]
        xT = np.ascontiguousarray(
            xn16[:, :, 1:].transpose(0, 2, 1)
            .reshape(NG, GROUP, D, TILE_N).transpose(0, 2, 1, 3)
            .reshape(NG * D, GROUP * TILE_N))
        # xn groups: [NG*TILE_N, GROUP*CW], row p = concat_j tile(g*8+j)[p, :]
        xng = np.ascontiguousarray(
            xn16.reshape(NG, GROUP, TILE_N, CW).transpose(0, 2, 1, 3)
            .reshape(NG * TILE_N, GROUP * CW))
        # bcols whole-core: [128, NT]
        ball = np.ascontiguousarray(bl.reshape(NT, TILE_N).T)
        in_maps.append({"xT": xT, "xn": xng, "ball": ball})

    consts = {
        "w1": np.ascontiguousarray(W1.astype(_bf16)),                      # [128, 64]
        "w2c": np.ascontiguousarray(
            np.concatenate([W2, W2], axis=0).astype(_bf16)),               # [128, 1]
        "b1c": np.ascontiguousarray(
            np.concatenate([b1, b1])[:, None].astype(np.float32)),         # [128, 1]
        "b2c": np.full((TILE_N, 1), float(b2[0]), dtype=np.float32),       # [128, 1]
        "iotab": np.broadcast_to(
            np.arange(TILE_N, dtype=np.float32), (TILE_N, TILE_N)
        ).astype(_bf16).copy(),                                            # [128, 128]
    }
    for m in in_maps:
        m.update(consts)
    return in_maps, K, float(b2[0])


def _build(K, b2f):
    import concourse.bass as bass
    import concourse.bacc as bacc
    import concourse.mybir as mybir
    from concourse.tile import TileContext

    dt = mybir.dt
    f32, bf16 = dt.float32, dt.bfloat16
    Alu = mybir.AluOpType
    Act = mybir.ActivationFunctionType

    NT = BLOCKS_PER_CORE * K
    NG = NT // GROUP
    GPB = K // GROUP

    nc = bacc.Bacc("TRN2", target_bir_lowering=False)
    xT_d = nc.dram_tensor("xT", [NG * D, GROUP * TILE_N], bf16, kind="ExternalInput")
    xn_d = nc.dram_tensor("xn", [NG * TILE_N, GROUP * CW], bf16, kind="ExternalInput")
    ba_d = nc.dram_tensor("ball", [TILE_N, NT], f32, kind="ExternalInput")
    w1_d = nc.dram_tensor("w1", [D, H], bf16, kind="ExternalInput")
    w2_d = nc.dram_tensor("w2c", [TILE_N, 1], bf16, kind="ExternalInput")
    b1_d = nc.dram_tensor("b1c", [TILE_N, 1], f32, kind="ExternalInput")
    b2_d = nc.dram_tensor("b2c", [TILE_N, 1], f32, kind="ExternalInput")
    io_d = nc.dram_tensor("iotab", [TILE_N, TILE_N], bf16, kind="ExternalInput")
    out_d = nc.dram_tensor("out", [SEGS_PER_CORE, D], f32, kind="ExternalOutput")

    xT_v = xT_d[:].rearrange("(g p) c -> g p c", p=D)
    xn_v = xn_d[:].rearrange("(g p) c -> g p c", p=TILE_N)

    with TileContext(nc) as tc:
        import contextlib
        ctx = contextlib.ExitStack()
        with ctx:
            cpool = ctx.enter_context(tc.tile_pool(name="consts", bufs=1))
            w1_s = cpool.tile([D, H], bf16, tag="w1")
            w2_s = cpool.tile([TILE_N, 1], bf16, tag="w2")
            b1_s = cpool.tile([TILE_N, 1], f32, tag="b1")
            b2_s = cpool.tile([TILE_N, 1], f32, tag="b2")
            io_s = cpool.tile([TILE_N, TILE_N], bf16, tag="io")
            ba_s = cpool.tile([TILE_N, NT], f32, tag="ba")
            nc.sync.dma_start(w1_s[:], w1_d[:])
            nc.sync.dma_start(w2_s[:], w2_d[:])
            nc.sync.dma_start(b1_s[:], b1_d[:])
            nc.sync.dma_start(b2_s[:], b2_d[:])
            nc.sync.dma_start(io_s[:], io_d[:])
            nc.sync.dma_start(ba_s[:], ba_d[:])

            xg_pool = ctx.enter_context(tc.tile_pool(name="xg", bufs=4))
            xn_pool = ctx.enter_context(tc.tile_pool(name="xnp", bufs=4))
            rhs_pool = ctx.enter_context(tc.tile_pool(name="rhs", bufs=3))
            hsb_pool = ctx.enter_context(tc.tile_pool(name="hsb", bufs=3))
            oh_pool = ctx.enter_context(tc.tile_pool(name="oh", bufs=8))
            ob_pool = ctx.enter_context(tc.tile_pool(name="ob", bufs=2))
            dn_pool = ctx.enter_context(tc.tile_pool(name="dn", bufs=2))
            ec_pool = ctx.enter_context(tc.tile_pool(name="ec", bufs=3))

            hps_pool = ctx.enter_context(tc.tile_pool(name="hps", bufs=3, space="PSUM"))
            sps_pool = ctx.enter_context(tc.tile_pool(name="sps", bufs=3, space="PSUM"))
            pps_pool = ctx.enter_context(tc.tile_pool(name="pps", bufs=2, space="PSUM"))

            for b in range(BLOCKS_PER_CORE):
                pps = pps_pool.tile([TILE_N, 129], f32, tag="pps")
                for g in range(GPB):
                    gg = b * GPB + g
                    t0 = gg * GROUP
                    xg = xg_pool.tile([D, GROUP * TILE_N], bf16, tag="xg")
                    nc.sync.dma_start(xg[:], xT_v[gg, :, :])
                    xn = xn_pool.tile([TILE_N, GROUP * CW], bf16, tag="xn")
                    nc.sync.dma_start(xn[:], xn_v[gg, :, :])

                    # ---- MLP scores ----
                    hps = hps_pool.tile([TILE_N, 4 * TILE_N], f32, tag="hps")
                    for half in range(2):
                        nc.tensor.matmul(
                            hps[half * H:(half + 1) * H, :],
                            w1_s[:],
                            xg[:, half * 512:(half + 1) * 512],
                            start=True, stop=True,
                        )
                    hsb = hsb_pool.tile([TILE_N, 4 * TILE_N], bf16, tag="hsb")
                    nc.scalar.activation(hsb[:], hps[:], Act.Prelu,
                                         bias=b1_s[:], scale=1.0, alpha=NEG_SLOPE)
                    sps = sps_pool.tile([TILE_N, GROUP], f32, tag="sps")
                    # interleave halves so adjacent mm2s hit distinct PE row-groups
                    for j in (0, 4, 1, 5, 2, 6, 3, 7):
                        half, q = divmod(j, 4)
                        nc.tensor.matmul(
                            sps[:, j:j + 1],
                            hsb[half * H:(half + 1) * H, q * TILE_N:(q + 1) * TILE_N],
                            w2_s[half * H:(half + 1) * H, :],
                            start=True, stop=True,
                        )
                    ecol = ec_pool.tile([TILE_N, GROUP], f32, tag="ecol")
                    nc.scalar.activation(ecol[:], sps[:], Act.Exp, bias=b2_s[:], scale=1.0)

                    # ---- pooling ----
                    rhs = rhs_pool.tile([TILE_N, GROUP * CW], bf16, tag="rhs")
                    for j in range(GROUP):
                        oh = oh_pool.tile([TILE_N, TILE_N], bf16, tag="oh")
                        oh_eng = nc.vector if (j % 2 == 0) else nc.gpsimd
                        oh_eng.tensor_scalar(
                            oh[:], io_s[:], ba_s[:, t0 + j:t0 + j + 1], None,
                            op0=Alu.is_equal)
                        nc.vector.tensor_scalar(
                            rhs[:, j * CW:(j + 1) * CW], xn[:, j * CW:(j + 1) * CW],
                            ecol[:, j:j + 1], None, op0=Alu.mult)
                        nc.tensor.matmul(
                            pps[:],
                            oh[:],
                            rhs[:, j * CW:(j + 1) * CW],
                            start=(g == 0 and j == 0),
                            stop=(g == GPB - 1 and j == GROUP - 1),
                        )

                # ---- flush block ----
                dn = dn_pool.tile([TILE_N, 1], f32, tag="dn")
                nc.vector.tensor_scalar(dn[:], pps[:, 0:1], 1e-16, None, op0=Alu.add)
                rc = dn_pool.tile([TILE_N, 1], f32, tag="rc")
                nc.vector.reciprocal(rc[:], dn[:])
                ob = ob_pool.tile([TILE_N, D], f32, tag="ob")
                nc.vector.tensor_scalar(ob[:], pps[:, 1:129], rc[:], None, op0=Alu.mult)
                nc.sync.dma_start(out_d[b * TILE_N:(b + 1) * TILE_N, :], ob[:])

    nc.compile()
    return nc


def kernel(**inputs):
    x = np.asarray(inputs["x"], dtype=np.float32)
    batch = np.asarray(inputs["batch"]).astype(np.int64)
    W1 = np.asarray(inputs["W1"], dtype=np.float32)
    b1 = np.asarray(inputs["b1"], dtype=np.float32)
    W2 = np.asarray(inputs["W2"], dtype=np.float32)
    b2 = np.asarray(inputs["b2"], dtype=np.float32)

    in_maps, K, b2f = _host_prep(x, batch, W1, b1, W2, b2)
    nc = _build(K, b2f)

    from concourse.bass_utils import run_bass_kernel_spmd
    res = run_bass_kernel_spmd(nc, in_maps, core_ids=list(range(NCORES)))
    out = np.concatenate([r["out"] for r in res.results], axis=0)
    return out.astype(np.float32)


# revision 14
# speedup vs baseline: 25.6599x; 25.6599x over previous
"""AttentionPooling (segment softmax + weighted segment-sum) Trainium2 kernel.

Algorithm (reference without explicit seg_max subtraction — scores are tiny,
|s| < ~3, so exp() is numerically safe unshifted and softmax is
shift-invariant):

    s_i   = W2^T prelu(W1^T x_i + b1) + b2          (per node)
    e_i   = exp(s_i)
    out_g = (sum_{i in g} e_i x_i) / (sum_{i in g} e_i + 1e-16)

Sharding: 16384 segments -> 8 cores x 16 blocks x 128 segments. batch is
sorted, so each (core, block) owns a contiguous node range; host pads every
block to the same number K of 128-node tiles so one SPMD program serves all
cores.

The host supplies each 128-node tile in BOTH orientations (bf16):
  - xT  [D=128, nodes]   feeds mm1 directly (no on-device transpose)
  - xn  [nodes, 1+D]     col 0 = 1.0; one DVE tensor_scalar produces the
                         pooling rhs [e | e*x] in a single op
Per tile: mm1 (PE) -> prelu (ACT, same table set as Exp) -> mm2 (PE, scores
column) -> exp (ACT) -> xe (DVE) -> one-hot pooling matmul (PE) accumulated
per 128-segment block in PSUM; block flush = reciprocal + scale + DMA out.
"""

import os
import numpy as np
import ml_dtypes

N = 2_000_000
D = 128
H = 64
G = 16384
NEG_SLOPE = 0.01
NCORES = 8
SEGS_PER_CORE = G // NCORES          # 2048
SEGS_PER_BLOCK = 128
BLOCKS_PER_CORE = SEGS_PER_CORE // SEGS_PER_BLOCK   # 16
TILE_N = 128
GROUP = 8                            # tiles per group (ACT chunking)
CW = D + 1                           # natural-tile width incl. ones column

_bf16 = ml_dtypes.bfloat16


def _host_prep(x, batch, W1, b1, W2, b2):
    bounds = np.searchsorted(batch, np.arange(0, G + 1, SEGS_PER_BLOCK))
    cnts = np.diff(bounds)
    K = int(np.max((cnts + TILE_N - 1) // TILE_N))
    K = ((K + GROUP - 1) // GROUP) * GROUP
    NT = BLOCKS_PER_CORE * K
    NG = NT // GROUP

    in_maps = []
    for c in range(NCORES):
        xn = np.zeros((NT, TILE_N, CW), dtype=np.float32)   # [tile, node, 1+D]
        bl = np.full((NT * TILE_N,), -1.0, dtype=np.float32)
        for b in range(BLOCKS_PER_CORE):
            gb = c * BLOCKS_PER_CORE + b
            lo, hi = int(bounds[gb]), int(bounds[gb + 1])
            n = hi - lo
            base = b * K * TILE_N
            blk = np.zeros((K * TILE_N, D), dtype=np.float32)
            blk[:n] = x[lo:hi]
            xn[b * K:(b + 1) * K, :, 1:] = blk.reshape(K, TILE_N, D)
            xn[b * K:(b + 1) * K, :, 0] = 1.0
            bl[base:base + n] = (batch[lo:hi] - gb * SEGS_PER_BLOCK).astype(np.float32)
        xn16 = xn.astype(_bf16)
        # xT groups: [NG*D, GROUP*TILE_N], row p = concat_j tileT(g*8+j)[p, :]
        xT = np.ascontiguousarray(
            xn16[:, :, 1:].transpose(0, 2, 1)
            .reshape(NG, GROUP, D, TILE_N).transpose(0, 2, 1, 3)
            .reshape(NG * D, GROUP * TILE_N))
        # xn groups: [NG*TILE_N, GROUP*CW], row p = concat_j tile(g*8+j)[p, :]
        xng = np.ascontiguousarray(
            xn16.reshape(NG, GROUP, TILE_N, CW).transpose(0, 2, 1, 3)
            .reshape(NG * TILE_N, GROUP * CW))
        # bcols whole-core: [128, NT]
        ball = np.ascontiguousarray(bl.reshape(NT, TILE_N).T)
        in_maps.append({"xT": xT, "xn": xng, "ball": ball})

    consts = {
        "w1": np.ascontiguousarray(W1.astype(_bf16)),                      # [128, 64]
        "w2c": np.ascontiguousarray(
            np.concatenate([W2, W2], axis=0).astype(_bf16)),               # [128, 1]
        "b1c": np.ascontiguousarray(
            np.concatenate([b1, b1])[:, None].astype(np.float32)),         # [128, 1]
        "b2c": np.full((TILE_N, 1), float(b2[0]), dtype=np.float32),       # [128, 1]
        "iotab": np.broadcast_to(
            np.arange(TILE_N, dtype=np.float32), (TILE_N, TILE_N)
        ).astype(_bf16).copy(),                                            # [128, 128]
    }
    for m in in_maps:
        m.update(consts)
    return in_maps, K, float(b2[0])


def _build(K, b2f):
    import concourse.bass as bass
    import concourse.bacc as bacc
    import concourse.mybir as mybir
    from concourse.tile import TileContext

    dt = mybir.dt
    f32, bf16 = dt.float32, dt.bfloat16
    Alu = mybir.AluOpType
    Act = mybir.ActivationFunctionType

    NT = BLOCKS_PER_CORE * K
    NG = NT // GROUP
    GPB = K // GROUP

    nc = bacc.Bacc("TRN2", target_bir_lowering=False)
    xT_d = nc.dram_tensor("xT", [NG * D, GROUP * TILE_N], bf16, kind="ExternalInput")
    xn_d = nc.dram_tensor("xn", [NG * TILE_N, GROUP * CW], bf16, kind="ExternalInput")
    ba_d = nc.dram_tensor("ball", [TILE_N, NT], f32, kind="ExternalInput")
    w1_d = nc.dram_tensor("w1", [D, H], bf16, kind="ExternalInput")
    w2_d = nc.dram_tensor("w2c", [TILE_N, 1], bf16, kind="ExternalInput")
    b1_d = nc.dram_tensor("b1c", [TILE_N, 1], f32, kind="ExternalInput")
    b2_d = nc.dram_tensor("b2c", [TILE_N, 1], f32, kind="ExternalInput")
    io_d = nc.dram_tensor("iotab", [TILE_N, TILE_N], bf16, kind="ExternalInput")
    out_d = nc.dram_tensor("out", [SEGS_PER_CORE, D], f32, kind="ExternalOutput")

    xT_v = xT_d[:].rearrange("(g p) c -> g p c", p=D)
    xn_v = xn_d[:].rearrange("(g p) c -> g p c", p=TILE_N)

    with TileContext(nc) as tc:
        import contextlib
        ctx = contextlib.ExitStack()
        with ctx:
            cpool = ctx.enter_context(tc.tile_pool(name="consts", bufs=1))
            w1_s = cpool.tile([D, H], bf16, tag="w1")
            w2_s = cpool.tile([TILE_N, 1], bf16, tag="w2")
            b1_s = cpool.tile([TILE_N, 1], f32, tag="b1")
            b2_s = cpool.tile([TILE_N, 1], f32, tag="b2")
            io_s = cpool.tile([TILE_N, TILE_N], bf16, tag="io")
            ba_s = cpool.tile([TILE_N, NT], f32, tag="ba")
            nc.sync.dma_start(w1_s[:], w1_d[:])
            nc.sync.dma_start(w2_s[:], w2_d[:])
            nc.sync.dma_start(b1_s[:], b1_d[:])
            nc.sync.dma_start(b2_s[:], b2_d[:])
            nc.sync.dma_start(io_s[:], io_d[:])
            nc.sync.dma_start(ba_s[:], ba_d[:])

            xg_pool = ctx.enter_context(tc.tile_pool(name="xg", bufs=4))
            xn_pool = ctx.enter_context(tc.tile_pool(name="xnp", bufs=4))
            rhs_pool = ctx.enter_context(tc.tile_pool(name="rhs", bufs=3))
            hsb_pool = ctx.enter_context(tc.tile_pool(name="hsb", bufs=3))
            oh_pool = ctx.enter_context(tc.tile_pool(name="oh", bufs=8))
            ob_pool = ctx.enter_context(tc.tile_pool(name="ob", bufs=2))
            dn_pool = ctx.enter_context(tc.tile_pool(name="dn", bufs=2))
            ec_pool = ctx.enter_context(tc.tile_pool(name="ec", bufs=3))

            hps_pool = ctx.enter_context(tc.tile_pool(name="hps", bufs=3, space="PSUM"))
            sps_pool = ctx.enter_context(tc.tile_pool(name="sps", bufs=3, space="PSUM"))
            pps_pool = ctx.enter_context(tc.tile_pool(name="pps", bufs=2, space="PSUM"))

            repeat = int(os.environ.get("BASSK_REPEAT", "1"))
            if repeat > 1:
                rloop = ctx.enter_context(tc.For_i(0, repeat, 1))
            for b in range(BLOCKS_PER_CORE):
                pps = pps_pool.tile([TILE_N, 129], f32, tag="pps")
                for g in range(GPB):
                    gg = b * GPB + g
                    t0 = gg * GROUP
                    xg = xg_pool.tile([D, GROUP * TILE_N], bf16, tag="xg")
                    nc.sync.dma_start(xg[:], xT_v[gg, :, :])
                    xn = xn_pool.tile([TILE_N, GROUP * CW], bf16, tag="xn")
                    nc.sync.dma_start(xn[:], xn_v[gg, :, :])

                    # ---- MLP scores ----
                    hps = hps_pool.tile([TILE_N, 4 * TILE_N], f32, tag="hps")
                    for half in range(2):
                        nc.tensor.matmul(
                            hps[half * H:(half + 1) * H, :],
                            w1_s[:],
                            xg[:, half * 512:(half + 1) * 512],
                            start=True, stop=True,
                        )
                    hsb = hsb_pool.tile([TILE_N, 4 * TILE_N], bf16, tag="hsb")
                    nc.scalar.activation(hsb[:], hps[:], Act.Prelu,
                                         bias=b1_s[:], scale=1.0, alpha=NEG_SLOPE)
                    sps = sps_pool.tile([TILE_N, GROUP], f32, tag="sps")
                    # interleave halves so adjacent mm2s hit distinct PE row-groups
                    for j in (0, 4, 1, 5, 2, 6, 3, 7):
                        half, q = divmod(j, 4)
                        nc.tensor.matmul(
                            sps[:, j:j + 1],
                            hsb[half * H:(half + 1) * H, q * TILE_N:(q + 1) * TILE_N],
                            w2_s[half * H:(half + 1) * H, :],
                            start=True, stop=True,
                        )
                    ecol = ec_pool.tile([TILE_N, GROUP], f32, tag="ecol")
                    nc.scalar.activation(ecol[:], sps[:], Act.Exp, bias=b2_s[:], scale=1.0)

                    # ---- pooling ----
                    rhs = rhs_pool.tile([TILE_N, GROUP * CW], bf16, tag="rhs")
                    for j in range(GROUP):
                        oh = oh_pool.tile([TILE_N, TILE_N], bf16, tag="oh")
                        oh_eng = nc.vector if (j % 2 == 0) else nc.gpsimd
                        oh_eng.tensor_scalar(
                            oh[:], io_s[:], ba_s[:, t0 + j:t0 + j + 1], None,
                            op0=Alu.is_equal)
                        nc.vector.tensor_scalar(
                            rhs[:, j * CW:(j + 1) * CW], xn[:, j * CW:(j + 1) * CW],
                            ecol[:, j:j + 1], None, op0=Alu.mult)
                        nc.tensor.matmul(
                            pps[:],
                            oh[:],
                            rhs[:, j * CW:(j + 1) * CW],
                            start=(g == 0 and j == 0),
                            stop=(g == GPB - 1 and j == GROUP - 1),
                        )

                # ---- flush block ----
                dn = dn_pool.tile([TILE_N, 1], f32, tag="dn")
                nc.vector.tensor_scalar(dn[:], pps[:, 0:1], 1e-16, None, op0=Alu.add)
                rc = dn_pool.tile([TILE_N, 1], f32, tag="rc")
                nc.vector.reciprocal(rc[:], dn[:])
                ob = ob_pool.tile([TILE_N, D], f32, tag="ob")
                nc.vector.tensor_scalar(ob[:], pps[:, 1:129], rc[:], None, op0=Alu.mult)
                nc.sync.dma_start(out_d[b * TILE_N:(b + 1) * TILE_N, :], ob[:])

    nc.compile()
    return nc


def kernel(**inputs):
    x = np.asarray(inputs["x"], dtype=np.float32)
    batch = np.asarray(inputs["batch"]).astype(np.int64)
    W1 = np.asarray(inputs["W1"], dtype=np.float32)
    b1 = np.asarray(inputs["b1"], dtype=np.float32)
    W2 = np.asarray(inputs["W2"], dtype=np.float32)
    b2 = np.asarray(inputs["b2"], dtype=np.float32)

    in_maps, K, b2f = _host_prep(x, batch, W1, b1, W2, b2)
    nc = _build(K, b2f)

    from concourse.bass_utils import run_bass_kernel_spmd
    res = run_bass_kernel_spmd(nc, in_maps, core_ids=list(range(NCORES)))
    out = np.concatenate([r["out"] for r in res.results], axis=0)
    return out.astype(np.float32)


# revision 17
# speedup vs baseline: 25.8083x; 1.0058x over previous
"""AttentionPooling (segment softmax + weighted segment-sum) Trainium2 kernel.

Algorithm (reference without explicit seg_max subtraction — scores are tiny,
|s| < ~3, so exp() is numerically safe unshifted and softmax is
shift-invariant):

    s_i   = W2^T prelu(W1^T x_i + b1) + b2          (per node)
    e_i   = exp(s_i)
    out_g = (sum_{i in g} e_i x_i) / (sum_{i in g} e_i + 1e-16)

Sharding: 16384 segments -> 8 cores x 16 blocks x 128 segments. batch is
sorted, so each (core, block) owns a contiguous node range; host pads every
block to the same number K of 128-node tiles so one SPMD program serves all
cores.

The host supplies each 128-node tile in BOTH orientations (bf16):
  - xT  [D=128, nodes]   feeds mm1 directly (no on-device transpose)
  - xn  [nodes, 1+D]     col 0 = 1.0; one DVE tensor_scalar produces the
                         pooling rhs [e | e*x] in a single op
Per tile: mm1 (PE) -> prelu (ACT, same table set as Exp) -> mm2 (PE, scores
column) -> exp (ACT) -> xe (DVE) -> one-hot pooling matmul (PE) accumulated
per 128-segment block in PSUM; block flush = reciprocal + scale + DMA out.
"""

import os
import numpy as np
import ml_dtypes

N = 2_000_000
D = 128
H = 64
G = 16384
NEG_SLOPE = 0.01
NCORES = 8
SEGS_PER_CORE = G // NCORES          # 2048
SEGS_PER_BLOCK = 128
BLOCKS_PER_CORE = SEGS_PER_CORE // SEGS_PER_BLOCK   # 16
TILE_N = 128
GROUP = 8                            # tiles per group (ACT chunking)
CW = D + 1                           # natural-tile width incl. ones column

_bf16 = ml_dtypes.bfloat16


def _host_prep(x, batch, W1, b1, W2, b2):
    bounds = np.searchsorted(batch, np.arange(0, G + 1, SEGS_PER_BLOCK))
    cnts = np.diff(bounds)
    K = int(np.max((cnts + TILE_N - 1) // TILE_N))
    K = ((K + GROUP - 1) // GROUP) * GROUP
    NT = BLOCKS_PER_CORE * K
    NG = NT // GROUP

    in_maps = []
    for c in range(NCORES):
        xn = np.zeros((NT, TILE_N, CW), dtype=np.float32)   # [tile, node, 1+D]
        bl = np.full((NT * TILE_N,), -1.0, dtype=np.float32)
        for b in range(BLOCKS_PER_CORE):
            gb = c * BLOCKS_PER_CORE + b
            lo, hi = int(bounds[gb]), int(bounds[gb + 1])
            n = hi - lo
            base = b * K * TILE_N
            blk = np.zeros((K * TILE_N, D), dtype=np.float32)
            blk[:n] = x[lo:hi]
            xn[b * K:(b + 1) * K, :, 1:] = blk.reshape(K, TILE_N, D)
            xn[b * K:(b + 1) * K, :, 0] = 1.0
            bl[base:base + n] = (batch[lo:hi] - gb * SEGS_PER_BLOCK).astype(np.float32)
        xn16 = xn.astype(_bf16)
        # xT groups: [NG*D, GROUP*TILE_N], row p = concat_j tileT(g*8+j)[p, :]
        xT = np.ascontiguousarray(
            xn16[:, :, 1:].transpose(0, 2, 1)
            .reshape(NG, GROUP, D, TILE_N).transpose(0, 2, 1, 3)
            .reshape(NG * D, GROUP * TILE_N))
        # xn groups: [NG*TILE_N, GROUP*CW], row p = concat_j tile(g*8+j)[p, :]
        xng = np.ascontiguousarray(
            xn16.reshape(NG, GROUP, TILE_N, CW).transpose(0, 2, 1, 3)
            .reshape(NG * TILE_N, GROUP * CW))
        # bcols whole-core: [128, NT]
        ball = np.ascontiguousarray(bl.reshape(NT, TILE_N).T)
        in_maps.append({"xT": xT, "xn": xng, "ball": ball})

    consts = {
        "w1": np.ascontiguousarray(
            np.concatenate([W1, W1], axis=1).astype(_bf16)),               # [128, 128]
        "w2c": np.ascontiguousarray(
            np.concatenate([W2, -NEG_SLOPE * W2], axis=0).astype(_bf16)),  # [128, 1]
        "b1c": np.ascontiguousarray(
            np.concatenate([b1, -b1])[:, None].astype(np.float32)),        # [128, 1]
        "scpm": np.ascontiguousarray(
            np.concatenate([np.ones(H), -np.ones(H)])[:, None]
            .astype(np.float32)),                                          # [128, 1]
        "b2c": np.full((TILE_N, 1), float(b2[0]), dtype=np.float32),       # [128, 1]
        "iotab": np.broadcast_to(
            np.arange(TILE_N, dtype=np.float32), (TILE_N, TILE_N)
        ).astype(_bf16).copy(),                                            # [128, 128]
    }
    for m in in_maps:
        m.update(consts)
    return in_maps, K, float(b2[0])


def _build(K, b2f):
    import concourse.bass as bass
    import concourse.bacc as bacc
    import concourse.mybir as mybir
    from concourse.tile import TileContext

    dt = mybir.dt
    f32, bf16 = dt.float32, dt.bfloat16
    Alu = mybir.AluOpType
    Act = mybir.ActivationFunctionType

    NT = BLOCKS_PER_CORE * K
    NG = NT // GROUP
    GPB = K // GROUP

    nc = bacc.Bacc("TRN2", target_bir_lowering=False)
    xT_d = nc.dram_tensor("xT", [NG * D, GROUP * TILE_N], bf16, kind="ExternalInput")
    xn_d = nc.dram_tensor("xn", [NG * TILE_N, GROUP * CW], bf16, kind="ExternalInput")
    ba_d = nc.dram_tensor("ball", [TILE_N, NT], f32, kind="ExternalInput")
    w1_d = nc.dram_tensor("w1", [D, 2 * H], bf16, kind="ExternalInput")
    sc_d = nc.dram_tensor("scpm", [TILE_N, 1], f32, kind="ExternalInput")
    w2_d = nc.dram_tensor("w2c", [TILE_N, 1], bf16, kind="ExternalInput")
    b1_d = nc.dram_tensor("b1c", [TILE_N, 1], f32, kind="ExternalInput")
    b2_d = nc.dram_tensor("b2c", [TILE_N, 1], f32, kind="ExternalInput")
    io_d = nc.dram_tensor("iotab", [TILE_N, TILE_N], bf16, kind="ExternalInput")
    out_d = nc.dram_tensor("out", [SEGS_PER_CORE, D], f32, kind="ExternalOutput")

    xT_v = xT_d[:].rearrange("(g p) c -> g p c", p=D)
    xn_v = xn_d[:].rearrange("(g p) c -> g p c", p=TILE_N)

    with TileContext(nc) as tc:
        import contextlib
        ctx = contextlib.ExitStack()
        with ctx:
            cpool = ctx.enter_context(tc.tile_pool(name="consts", bufs=1))
            w1_s = cpool.tile([D, 2 * H], bf16, tag="w1")
            sc_s = cpool.tile([TILE_N, 1], f32, tag="sc")
            w2_s = cpool.tile([TILE_N, 1], bf16, tag="w2")
            b1_s = cpool.tile([TILE_N, 1], f32, tag="b1")
            b2_s = cpool.tile([TILE_N, 1], f32, tag="b2")
            io_s = cpool.tile([TILE_N, TILE_N], bf16, tag="io")
            ba_s = cpool.tile([TILE_N, NT], f32, tag="ba")
            nc.sync.dma_start(w1_s[:], w1_d[:])
            nc.sync.dma_start(sc_s[:], sc_d[:])
            nc.sync.dma_start(w2_s[:], w2_d[:])
            nc.sync.dma_start(b1_s[:], b1_d[:])
            nc.sync.dma_start(b2_s[:], b2_d[:])
            nc.sync.dma_start(io_s[:], io_d[:])
            nc.sync.dma_start(ba_s[:], ba_d[:])

            xg_pool = ctx.enter_context(tc.tile_pool(name="xg", bufs=4))
            xn_pool = ctx.enter_context(tc.tile_pool(name="xnp", bufs=4))
            rhs_pool = ctx.enter_context(tc.tile_pool(name="rhs", bufs=3))
            hsb_pool = ctx.enter_context(tc.tile_pool(name="hsb", bufs=3))
            oh_pool = ctx.enter_context(tc.tile_pool(name="oh", bufs=8))
            ob_pool = ctx.enter_context(tc.tile_pool(name="ob", bufs=2))
            dn_pool = ctx.enter_context(tc.tile_pool(name="dn", bufs=2))
            ec_pool = ctx.enter_context(tc.tile_pool(name="ec", bufs=3))

            hps_pool = ctx.enter_context(tc.tile_pool(name="hps", bufs=3, space="PSUM"))
            sps_pool = ctx.enter_context(tc.tile_pool(name="sps", bufs=3, space="PSUM"))
            pps_pool = ctx.enter_context(tc.tile_pool(name="pps", bufs=2, space="PSUM"))

            repeat = int(os.environ.get("BASSK_REPEAT", "1"))
            if repeat > 1:
                rloop = ctx.enter_context(tc.For_i(0, repeat, 1))
            for b in range(BLOCKS_PER_CORE):
                pps = pps_pool.tile([TILE_N, 129], f32, tag="pps")
                for g in range(GPB):
                    gg = b * GPB + g
                    t0 = gg * GROUP
                    xg = xg_pool.tile([D, GROUP * TILE_N], bf16, tag="xg")
                    nc.sync.dma_start(xg[:], xT_v[gg, :, :])
                    xn = xn_pool.tile([TILE_N, GROUP * CW], bf16, tag="xn")
                    nc.sync.dma_start(xn[:], xn_v[gg, :, :])

                    # ---- MLP scores (lrelu = relu(z) - slope*relu(-z), stacked K) ----
                    sps = sps_pool.tile([TILE_N, GROUP], f32, tag="sps")
                    for half in range(2):
                        hps = hps_pool.tile([TILE_N, 4 * TILE_N], f32, tag="hps")
                        nc.tensor.matmul(
                            hps[:],
                            w1_s[:],
                            xg[:, half * 512:(half + 1) * 512],
                            start=True, stop=True,
                        )
                        hsb = hsb_pool.tile([TILE_N, 4 * TILE_N], bf16, tag="hsb")
                        nc.scalar.activation(hsb[:], hps[:], Act.Relu,
                                             bias=b1_s[:], scale=sc_s[:])
                        for q in range(4):
                            nc.tensor.matmul(
                                sps[:, half * 4 + q:half * 4 + q + 1],
                                hsb[:, q * TILE_N:(q + 1) * TILE_N],
                                w2_s[:],
                                start=True, stop=True,
                            )
                    ecol = ec_pool.tile([TILE_N, GROUP], f32, tag="ecol")
                    nc.scalar.activation(ecol[:], sps[:], Act.Exp, bias=b2_s[:], scale=1.0)

                    # ---- pooling ----
                    rhs = rhs_pool.tile([TILE_N, GROUP * CW], bf16, tag="rhs")
                    for j in range(GROUP):
                        oh = oh_pool.tile([TILE_N, TILE_N], bf16, tag="oh")
                        oh_eng = nc.vector if (j % 2 == 0) else nc.gpsimd
                        oh_eng.tensor_scalar(
                            oh[:], io_s[:], ba_s[:, t0 + j:t0 + j + 1], None,
                            op0=Alu.is_equal)
                        nc.vector.tensor_scalar(
                            rhs[:, j * CW:(j + 1) * CW], xn[:, j * CW:(j + 1) * CW],
                            ecol[:, j:j + 1], None, op0=Alu.mult)
                        nc.tensor.matmul(
                            pps[:],
                            oh[:],
                            rhs[:, j * CW:(j + 1) * CW],
                            start=(g == 0 and j == 0),
                            stop=(g == GPB - 1 and j == GROUP - 1),
                        )

                # ---- flush block ----
                dn = dn_pool.tile([TILE_N, 1], f32, tag="dn")
                nc.vector.tensor_scalar(dn[:], pps[:, 0:1], 1e-16, None, op0=Alu.add)
                rc = dn_pool.tile([TILE_N, 1], f32, tag="rc")
                nc.vector.reciprocal(rc[:], dn[:])
                ob = ob_pool.tile([TILE_N, D], f32, tag="ob")
                nc.vector.tensor_scalar(ob[:], pps[:, 1:129], rc[:], None, op0=Alu.mult)
                nc.sync.dma_start(out_d[b * TILE_N:(b + 1) * TILE_N, :], ob[:])

    nc.compile()
    return nc


def kernel(**inputs):
    x = np.asarray(inputs["x"], dtype=np.float32)
    batch = np.asarray(inputs["batch"]).astype(np.int64)
    W1 = np.asarray(inputs["W1"], dtype=np.float32)
    b1 = np.asarray(inputs["b1"], dtype=np.float32)
    W2 = np.asarray(inputs["W2"], dtype=np.float32)
    b2 = np.asarray(inputs["b2"], dtype=np.float32)

    in_maps, K, b2f = _host_prep(x, batch, W1, b1, W2, b2)
    nc = _build(K, b2f)

    from concourse.bass_utils import run_bass_kernel_spmd
    res = run_bass_kernel_spmd(nc, in_maps, core_ids=list(range(NCORES)))
    out = np.concatenate([r["out"] for r in res.results], axis=0)
    return out.astype(np.float32)
